# revision 45
# baseline (speedup 1.0000x reference)
"""Trainium2 Bass kernel for 16-head MHA (B=4, S=2048, E=1024), 8 NeuronCores.

Sharding: core c handles batch b = c//2 and head-group g = c%2 (8 heads each).
Column-parallel Wq/Wk/Wv, row-parallel Wo; the two partial Wo outputs per
batch are summed on the host.

All matmuls are fp16 with fp32 PSUM accumulation (fp8 variants tested ~3-10x
over the accuracy budget). The performance levers vs the naive schedule:
  - The PE ramps to full clock only after ~3us of gapless execution, so the
    emission order keeps PE dense: V/Q/K projections stream through two
    40KB pack buffers, remaining Q/K projection chunks are interleaved as
    filler groups between attention key blocks.
  - exp is split across ScalarE (12/16 tiles, exact Exp) and DVE (4/16
    tiles, Schraudolph bitcast-exp: one tensor_scalar affine fp32->uint16
    whose bits are the f16 exponential; ~+-3% on those tiles only), sized so
    the exp stream stays just under the PE's B-phase time.
  - The [65, 1024] ctx+denominator psum is copied by the Pool engine into
    per-head staging; denominator rows are gathered by SBUF-SBUF DMA.
  - Phase C reciprocal on DVE; phase D staged per-eb with copies on ScalarE
    and outputs shipped on 3 DMA rings as they finish.
"""

import sys

sys.path.insert(0, "/opt/trn_rl_repo")

import numpy as np

# Problem constants (hardcoded; kernel.py must be self-contained).
B = 4
S = 2048
E = 1024
H = 16
D = 64
N_CORES = 8
HL = H // 2  # heads per core
O = HL * D  # 512 local out features of q/k/v projections
IC = E // 128  # 8 contraction chunks
OB = O // 128  # 4 output row-blocks (head pairs)
TB = S // 128  # 16 token blocks
KB = S // 128  # 16 key blocks per head
QCHUNK = 1024
QC = S // QCHUNK  # 2
NV = D + 1  # v dims + ones column

DVE_KB = (3, 7, 11, 15)  # key blocks whose exp runs on DVE (Schraudolph)
A16 = 1024.0 / np.log(2.0)  # Schraudolph slope for f16 bits
# f16 exponent offset, minus the PWL centering constant, plus 0.5 because the
# DVE float->uint convert truncates (verified) rather than rounds.
B16 = 15.0 * 1024.0 - 60.0 + 0.5

_CACHE = {}


def _build(phases="ABCD"):
    import concourse.bass as bass
    import concourse.mybir as mybir
    from concourse import bacc, tile

    f32 = mybir.dt.float32
    f16 = mybir.dt.float16
    u16 = mybir.dt.uint16
    Exp = mybir.ActivationFunctionType.Exp

    nc = bacc.Bacc(None, target_bir_lowering=False)

    # Packed inputs: per contraction chunk ic, activation chunk [128, S] then
    # projection-weight chunk [128, O]; consumed V -> Q -> K through two pack
    # buffers.
    XW = S + O
    INQ = nc.dram_tensor("INQ", [128, IC * XW], f16, kind="ExternalInput")
    INK = nc.dram_tensor("INK", [128, IC * XW], f16, kind="ExternalInput")
    INV = nc.dram_tensor("INV", [128, IC * XW], f16, kind="ExternalInput")
    INW = nc.dram_tensor("INW", [128, OB * E], f16, kind="ExternalInput")
    SEL = nc.dram_tensor("SEL", [1, 2, 128], f16, kind="ExternalInput")
    OUT = nc.dram_tensor("OUT", [E, S], f16, kind="ExternalOutput")

    with tile.TileContext(nc) as tc:
        with (
            tc.tile_pool(name="consts", bufs=1) as constp,
            tc.tile_pool(name="weights", bufs=1) as wp,
            tc.tile_pool(name="qkv", bufs=1) as qkvp,
            tc.tile_pool(name="pack", bufs=1) as packp,
        ):
            sel_sb = constp.tile([1, 2, 128], f16, tag="sel")
            wo_sb = wp.tile([128, OB, E], f16, tag="wo")

            qt_sb = [qkvp.tile([128, S], f16, tag=f"qt{ob}", name=f"qt{ob}") for ob in range(OB)]
            kt_sb = [qkvp.tile([128, S], f16, tag=f"kt{ob}", name=f"kt{ob}") for ob in range(OB)]
            v_sb = [qkvp.tile([128, HL * NV], f16, tag=f"v{tb}", name=f"v{tb}") for tb in range(TB)]
            for tb in range(TB):
                ones = v_sb[tb].rearrange("p (h x) -> p h x", x=NV)[:, :, D : D + 1]
                nc.vector.memset(ones, 1.0)

            # Per-chunk DMAs: subtile dep tracking lets the first projection
            # matmul start as soon as its ic-chunk lands, not the whole pack.
            pkv = packp.tile([128, IC, XW], f16, tag="pkv", name="pkV")
            pkq = packp.tile([128, IC, XW], f16, tag="pkq", name="pkQ")
            pkk = packp.tile([128, IC, XW], f16, tag="pkk", name="pkK")
            for ic in range(IC):
                nc.gpsimd.dma_start(pkv[:, ic, :], INV[:, ic * XW : (ic + 1) * XW])
                nc.sync.dma_start(pkq[:, ic, :], INQ[:, ic * XW : (ic + 1) * XW])
                nc.scalar.dma_start(pkk[:, ic, :], INK[:, ic * XW : (ic + 1) * XW])
            nc.sync.dma_start(wo_sb[:].rearrange("p a b -> p (a b)"), INW[:])
            nc.scalar.dma_start(sel_sb[:], SEL[:])

            skip_proj = "Y" in phases

            # PSUM (8 banks): scores/proj shared ring 3x[128,1024] (6) + ctx
            # accumulator 1x[65,1024] (2). Projection psums borrow score
            # ring slots so the score stream keeps a 3-deep lookahead.
            with (
                tc.tile_pool(name="attn", bufs=4) as attnp,
                tc.tile_pool(name="psum_s", bufs=3, space="PSUM") as pss,
                tc.tile_pool(name="psum_c", bufs=1, space="PSUM") as psc,
                tc.tile_pool(name="norm", bufs=2) as normp,
            ):
                ctxt_sb = [
                    qkvp.tile([128, S], f16, tag=f"ctxt{ob}", name=f"ctxt{ob}")
                    for ob in range(OB)
                ]

                def proj_qk_group(which, pk, dst, ob, j, ceng):
                    # one [128, 512] column group of the q/k projection
                    ps = pss.tile([128, 512], f32, tag="psS", name=f"ps_{which}{ob}_{j}")
                    for ic in range(IC):
                        nc.tensor.matmul(
                            ps[:],
                            pk[:, ic, S + ob * 128 : S + (ob + 1) * 128],
                            pk[:, ic, j * 512 : (j + 1) * 512],
                            start=(ic == 0),
                            stop=(ic == IC - 1),
                        )
                    if ceng is nc.scalar:
                        ceng.copy(dst[ob][:, j * 512 : (j + 1) * 512], ps[:])
                    else:
                        ceng.tensor_copy(dst[ob][:, j * 512 : (j + 1) * 512], ps[:])

                def proj_v_tb(tb):
                    if skip_proj:
                        return
                    ps = pss.tile([128, 512], f32, tag="psS", name=f"ps_v{tb}")
                    for ic in range(IC):
                        nc.tensor.matmul(
                            ps[:],
                            pkv[:, ic, tb * 128 : (tb + 1) * 128],
                            pkv[:, ic, S : S + O],
                            start=(ic == 0),
                            stop=(ic == IC - 1),
                        )
                    vdst = v_sb[tb].rearrange("p (h x) -> p h x", x=NV)[:, :, 0:D]
                    # Pool/GPSIMD cannot access PSUM (walrus verification)
                    nc.scalar.copy(vdst, ps[:].rearrange("p (h d) -> p h d", d=D))

                # filler queue: remaining q/k projection groups, emitted
                # between attention key blocks to keep the PE stream dense
                fillers = []
                if not skip_proj:
                    for ob in range(1, OB):
                        for which, pk, dst in (("q", pkq, qt_sb), ("k", pkk, kt_sb)):
                            for j in range(S // 512):
                                fillers.append(
                                    (which, pk, dst, ob, j)
                                )
                fill_i = [0]

                def emit_filler():
                    if fill_i[0] < len(fillers):
                        which, pk, dst, ob, j = fillers[fill_i[0]]
                        fill_i[0] += 1
                        proj_qk_group(which, pk, dst, ob, j, nc.vector)

                # Deferred per-chunk finalize (broadcast + normalize mul),
                # emitted early in the NEXT chunk so the waiting instructions
                # never head-of-line-block the PE/DVE queues.
                pending = [None]

                def flush_pending():
                    if pending[0] is not None:
                        fin, pending[0] = pending[0], None
                        fin()

                def attn_head(hl):
                    ob, r0 = hl // 2, (hl % 2) * 64
                    LAG = 3  # ctx matmuls trail scores by LAG key blocks
                    for qc in range(QC):
                        chunk = 2 * hl + qc
                        fill_at = (
                            ()
                            if chunk < 2
                            else (3, 7, 11, 14)
                            if chunk < 4
                            else (4, 9, 14)
                            if chunk < 8
                            else (8,)
                        )
                        q0 = qc * QCHUNK
                        pc = psc.tile([NV, QCHUNK], f32, tag="pc", name=f"pc{hl}_{qc}")
                        at_live = {}
                        for step in range(KB + LAG):
                            kb = step
                            if kb < KB:
                                if hl == 0 and qc == 0:
                                    # warm-up: stream remaining projections
                                    # between the first head's key blocks
                                    if kb in (4, 8, 12):
                                        proj_qk_group("k", pkk, kt_sb, 0, kb // 4, nc.scalar)
                                    if kb == 2:
                                        proj_qk_group("q", pkq, qt_sb, 0, 2, nc.scalar)
                                    if kb == 5:
                                        proj_qk_group("q", pkq, qt_sb, 0, 3, nc.scalar)
                                    proj_v_tb(kb)
                                ps = pss.tile(
                                    [128, QCHUNK], f32, tag="psS", name=f"sc{hl}_{qc}_{kb}"
                                )
                                at = attnp.tile(
                                    [128, QCHUNK], f16, tag="at", name=f"at{hl}_{qc}_{kb}"
                                )
                                for j in range(QCHUNK // 512):
                                    nc.tensor.matmul(
                                        ps[:, j * 512 : (j + 1) * 512],
                                        kt_sb[ob][r0 : r0 + 64, kb * 128 : (kb + 1) * 128],
                                        qt_sb[ob][r0 : r0 + 64, q0 + j * 512 : q0 + (j + 1) * 512],
                                        start=True,
                                        stop=True,
                                    )
                                if kb in DVE_KB:
                                    nc.vector.tensor_scalar(
                                        at[:].bitcast(u16),
                                        ps[:],
                                        A16 * 0.125,
                                        B16,
                                        mybir.AluOpType.mult,
                                        mybir.AluOpType.add,
                                    )
                                else:
                                    nc.scalar.activation(at[:], ps[:], Exp, scale=0.125)
                                at_live[kb] = at
                            ck = step - LAG
                            if ck >= 0:
                                at = at_live.pop(ck)
                                for j in range(QCHUNK // 512):
                                    nc.tensor.matmul(
                                        pc[:, j * 512 : (j + 1) * 512],
                                        v_sb[ck][:, hl * NV : (hl + 1) * NV],
                                        at[:, j * 512 : (j + 1) * 512],
                                        start=(ck == 0),
                                        stop=(ck == KB - 1),
                                    )
                            if step == 1:
                                flush_pending()
                            if step in fill_at:
                                emit_filler()
                        # reciprocal of the denominator row now; broadcast +
                        # multiply deferred into the next chunk
                        rec = normp.tile([1, QCHUNK], f16, tag="rec", name=f"rec{hl}_{qc}")
                        with nc.allow_low_precision(reason="f16 softmax reciprocal"):
                            nc.vector.reciprocal(rec[:], pc[64:65, :])

                        def finalize(pc=pc, rec=rec, q0=q0, hl=hl, ob=ob, r0=r0):
                            # vector ops may read at most one PSUM operand:
                            # stage the ctx rows to SBUF, then multiply by the
                            # broadcast reciprocal still in PSUM
                            stg = normp.tile([64, QCHUNK], f16, tag="stg", name=f"stg{hl}_{q0}", bufs=1)
                            nc.vector.tensor_copy(stg[:], pc[0:64, :])
                            pb = pss.tile([128, QCHUNK], f32, tag="psS", name=f"pb{hl}_{q0}")
                            for j in range(QCHUNK // 512):
                                nc.tensor.matmul(
                                    pb[:, j * 512 : (j + 1) * 512],
                                    sel_sb[0:1, hl % 2, :],
                                    rec[:, j * 512 : (j + 1) * 512],
                                    start=True,
                                    stop=True,
                                )
                            nc.vector.tensor_mul(
                                ctxt_sb[ob][r0 : r0 + 64, q0 : q0 + QCHUNK],
                                stg[:],
                                pb[r0 : r0 + 64, :],
                            )

                        pending[0] = finalize

                if not skip_proj:
                    for j in (0, 1):
                        proj_qk_group("q", pkq, qt_sb, 0, j, nc.scalar)
                    proj_qk_group("k", pkk, kt_sb, 0, 0, nc.scalar)
                if "B" in phases:
                    for hl in range(HL):
                        attn_head(hl)
                    flush_pending()
                else:
                    for tb in range(TB):
                        proj_v_tb(tb)
                    if not skip_proj:
                        for j in (2, 3):
                            proj_qk_group("q", pkq, qt_sb, 0, j, nc.scalar)
                        for j in (1, 2, 3):
                            proj_qk_group("k", pkk, kt_sb, 0, j, nc.scalar)
                while fill_i[0] < len(fillers):
                    emit_filler()

            # ============ Phase D: output projection ============
            with (
                tc.tile_pool(name="outs", bufs=2) as outsp,
                tc.tile_pool(name="psum_o", bufs=2, space="PSUM") as pso,
            ):
                outv = OUT[:].rearrange("(eb p) s -> p eb s", p=128)
                rings = (nc.sync, nc.scalar, nc.gpsimd)
                for eb in range(E // 128 if "D" in phases else 0):
                    po = pso.tile([128, S], f32, tag="po", name=f"po{eb}")
                    for oc in range(OB):
                        for j in range(S // 512):
                            nc.tensor.matmul(
                                po[:, j * 512 : (j + 1) * 512],
                                wo_sb[:, oc, eb * 128 : (eb + 1) * 128],
                                ctxt_sb[oc][:, j * 512 : (j + 1) * 512],
                                start=(oc == 0),
                                stop=(oc == OB - 1),
                            )
                    so = outsp.tile([128, S], f16, tag="so", name=f"so{eb}")
                    nc.scalar.copy(so[:], po[:])
                    rings[eb % 3].dma_start(outv[:, eb, :], so[:])

    nc.compile()
    return nc


def _get_nc():
    if "nc" not in _CACHE:
        _CACHE["nc"] = _build()
    return _CACHE["nc"]


def _shard_inputs(Q, K, V, Wq, Wk, Wv, Wo):
    f16 = np.float16
    Q = np.asarray(Q, np.float32)
    K = np.asarray(K, np.float32)
    V = np.asarray(V, np.float32)
    Wq = np.asarray(Wq, np.float32)
    Wk = np.asarray(Wk, np.float32)
    Wv = np.asarray(Wv, np.float32)
    Wo = np.asarray(Wo, np.float32)

    sel = np.zeros((1, 2, 128), np.float32)
    sel[0, 0, 0:64] = 1.0
    sel[0, 1, 64:128] = 1.0
    sel = sel.astype(f16)

    def pack_xw(XTb, WTg):
        # [128, IC*(S+O)]: per chunk ic, activation chunk then weight chunk
        parts = []
        for ic in range(IC):
            parts.append(XTb[ic * 128 : (ic + 1) * 128, :])
            parts.append(WTg[ic * 128 : (ic + 1) * 128, :])
        return np.ascontiguousarray(np.concatenate(parts, axis=1)).astype(f16)

    in_maps = []
    for c in range(N_CORES):
        b, g = divmod(c, 2)
        sl = slice(g * O, (g + 1) * O)
        wot = Wo[:, sl].T  # [O, E]
        inw = np.concatenate(
            [wot[oc * 128 : (oc + 1) * 128, :] for oc in range(OB)], axis=1
        )
        in_maps.append(
            {
                "SEL": sel,
                "INQ": pack_xw(Q[b].T, Wq[sl, :].T),
                "INK": pack_xw(K[b].T, Wk[sl, :].T),
                "INV": pack_xw(V[b].T, Wv[sl, :].T),
                "INW": np.ascontiguousarray(inw).astype(f16),
            }
        )
    return in_maps


class _Runner:
    """Compile-once executor for the SPMD bass program on 8 cores."""

    def __init__(self, nc, donate=True):
        import jax
        import concourse.mybir as mybir
        from concourse import bass2jax

        bass2jax.install_neuronx_cc_hook()
        self.jax = jax
        self.nc = nc
        partition_name = (
            nc.partition_id_tensor.name if nc.partition_id_tensor else None
        )
        in_names, out_names, out_avals = [], [], []
        for alloc in nc.m.functions[0].allocations:
            if not isinstance(alloc, mybir.MemoryLocationSet):
                continue
            name = alloc.memorylocations[0].name
            if alloc.kind == "ExternalInput":
                if name != partition_name:
                    in_names.append(name)
            elif alloc.kind == "ExternalOutput":
                out_names.append(name)
                out_avals.append(
                    jax.core.ShapedArray(
                        tuple(alloc.tensor_shape), mybir.dt.np(alloc.dtype)
                    )
                )
        self.in_names = in_names
        self.out_names = out_names
        self.out_avals = out_avals
        n_params = len(in_names)
        n_outs = len(out_names)
        all_in_names = list(in_names) + list(out_names)
        if partition_name is not None:
            all_in_names.append(partition_name)
        all_in_names = tuple(all_in_names)

        def _body(*args):
            operands = list(args)
            if partition_name is not None:
                operands.append(bass2jax.partition_id_tensor())
            outs = bass2jax._bass_exec_p.bind(
                *operands,
                out_avals=tuple(out_avals),
                in_names=all_in_names,
                out_names=tuple(out_names),
                lowering_input_output_aliases=(),
                sim_require_finite=True,
                sim_require_nnan=True,
                nc=nc,
            )
            return tuple(outs)

        from concourse.bass2jax import Mesh, PartitionSpec, shard_map

        devices = jax.devices()[:N_CORES]
        mesh = Mesh(np.asarray(devices), ("core",))
        self.mesh = mesh
        self.pspec = PartitionSpec("core")
        self.sharded = jax.jit(
            shard_map(
                _body,
                mesh=mesh,
                in_specs=(PartitionSpec("core"),) * (n_params + n_outs),
                out_specs=(PartitionSpec("core"),) * n_outs,
                check_rep=False,
            ),
            donate_argnums=(
                tuple(range(n_params, n_params + n_outs)) if donate else ()
            ),
            keep_unused=True,
        )

    def concat_inputs(self, in_maps):
        return [
            np.concatenate([np.asarray(m[name]) for m in in_maps], axis=0)
            for name in self.in_names
        ]

    def zero_outs(self):
        return [
            np.zeros((N_CORES * a.shape[0], *a.shape[1:]), a.dtype)
            for a in self.out_avals
        ]

    def __call__(self, concat_in, concat_zeros=None):
        if concat_zeros is None:
            concat_zeros = self.zero_outs()
        out_arrs = self.sharded(*concat_in, *concat_zeros)
        return [
            {
                name: np.asarray(out_arrs[i]).reshape(
                    N_CORES, *self.out_avals[i].shape
                )[c]
                for i, name in enumerate(self.out_names)
            }
            for c in range(N_CORES)
        ]


def _get_runner():
    if "runner" not in _CACHE:
        _CACHE["runner"] = _Runner(_get_nc())
    return _CACHE["runner"]


def kernel(Q, K, V, mask, Wq, Wk, Wv, Wo):
    runner = _get_runner()
    in_maps = _shard_inputs(Q, K, V, Wq, Wk, Wv, Wo)
    results = runner(runner.concat_inputs(in_maps))
    out = np.empty((B, S, E), np.float32)
    for b in range(B):
        acc = results[2 * b]["OUT"].astype(np.float32) + results[2 * b + 1][
            "OUT"
        ].astype(np.float32)
        out[b] = acc.T
    return out


# revision 47
# speedup vs baseline: 1.0864x; 1.0864x over previous
"""Trainium2 Bass kernel for 16-head MHA (B=4, S=2048, E=1024), 8 NeuronCores.

Sharding: core c handles batch b = c//2 and head-group g = c%2 (8 heads each).
Column-parallel Wq/Wk/Wv, row-parallel Wo; the two partial Wo outputs per
batch are summed on the host.

All matmuls are fp16 with fp32 PSUM accumulation (fp8 variants tested ~3-10x
over the accuracy budget). The performance levers vs the naive schedule:
  - The PE ramps to full clock only after ~3us of gapless execution, so the
    emission order keeps PE dense: V/Q/K projections stream through two
    40KB pack buffers, remaining Q/K projection chunks are interleaved as
    filler groups between attention key blocks.
  - exp is split across ScalarE (12/16 tiles, exact Exp) and DVE (4/16
    tiles, Schraudolph bitcast-exp: one tensor_scalar affine fp32->uint16
    whose bits are the f16 exponential; ~+-3% on those tiles only), sized so
    the exp stream stays just under the PE's B-phase time.
  - Per (head, 1024-query chunk) the softmax normalization runs straight
    out of PSUM: DVE reciprocal of the denominator row (ones-column trick),
    1-partition matmul broadcast, DVE stage+multiply into ctxt -- all
    deferred into the next chunk so waiting instructions never block the
    in-order engine queues.
  - Phase D staged per-eb with copies on ScalarE and outputs shipped on 3
    DMA rings as they finish. (Pool/GPSIMD cannot touch PSUM, and vector
    ops may read at most one PSUM operand -- walrus-verified rules.)
"""

import sys

sys.path.insert(0, "/opt/trn_rl_repo")

import numpy as np

# Problem constants (hardcoded; kernel.py must be self-contained).
B = 4
S = 2048
E = 1024
H = 16
D = 64
N_CORES = 8
HL = H // 2  # heads per core
O = HL * D  # 512 local out features of q/k/v projections
IC = E // 128  # 8 contraction chunks
OB = O // 128  # 4 output row-blocks (head pairs)
TB = S // 128  # 16 token blocks
KB = S // 128  # 16 key blocks per head
QCHUNK = 1024
QC = S // QCHUNK  # 2
NV = D + 1  # v dims + ones column

DVE_KB = (3, 7, 11, 15)  # key blocks whose exp runs on DVE (Schraudolph)
A16 = 1024.0 / np.log(2.0)  # Schraudolph slope for f16 bits
# f16 exponent offset, minus the PWL centering constant, plus 0.5 because the
# DVE float->uint convert truncates (verified) rather than rounds.
B16 = 15.0 * 1024.0 - 60.0 + 0.5

_CACHE = {}

DMA_MODE = "chunk3"  # "big3" | "chunk3"


def _build(phases="ABCD"):
    import concourse.bass as bass
    import concourse.mybir as mybir
    from concourse import bacc, tile

    f32 = mybir.dt.float32
    f16 = mybir.dt.float16
    u16 = mybir.dt.uint16
    Exp = mybir.ActivationFunctionType.Exp

    nc = bacc.Bacc(None, target_bir_lowering=False)

    # Packed inputs: per contraction chunk ic, activation chunk [128, S] then
    # projection-weight chunk [128, O]; consumed V -> Q -> K through two pack
    # buffers.
    XW = S + O
    INQ = nc.dram_tensor("INQ", [128, IC * XW], f16, kind="ExternalInput")
    INK = nc.dram_tensor("INK", [128, IC * XW], f16, kind="ExternalInput")
    INV = nc.dram_tensor("INV", [128, IC * XW], f16, kind="ExternalInput")
    INW = nc.dram_tensor("INW", [128, OB * E], f16, kind="ExternalInput")
    SEL = nc.dram_tensor("SEL", [1, 2, 128], f16, kind="ExternalInput")
    OUT = nc.dram_tensor("OUT", [E, S], f16, kind="ExternalOutput")

    with tile.TileContext(nc) as tc:
        with (
            tc.tile_pool(name="consts", bufs=1) as constp,
            tc.tile_pool(name="weights", bufs=1) as wp,
            tc.tile_pool(name="qkv", bufs=1) as qkvp,
            tc.tile_pool(name="pack", bufs=1) as packp,
        ):
            sel_sb = constp.tile([1, 2, 128], f16, tag="sel")
            wo_sb = wp.tile([128, OB, E], f16, tag="wo")

            qt_sb = [qkvp.tile([128, S], f16, tag=f"qt{ob}", name=f"qt{ob}") for ob in range(OB)]
            kt_sb = [qkvp.tile([128, S], f16, tag=f"kt{ob}", name=f"kt{ob}") for ob in range(OB)]
            v_sb = [qkvp.tile([128, HL * NV], f16, tag=f"v{tb}", name=f"v{tb}") for tb in range(TB)]
            for tb in range(TB):
                ones = v_sb[tb].rearrange("p (h x) -> p h x", x=NV)[:, :, D : D + 1]
                nc.vector.memset(ones, 1.0)

            # Per-chunk DMAs: subtile dep tracking lets the first projection
            # matmul start as soon as its ic-chunk lands, not the whole pack.
            pkv = packp.tile([128, IC, XW], f16, tag="pkv", name="pkV")
            pkq = packp.tile([128, IC, XW], f16, tag="pkq", name="pkQ")
            pkk = packp.tile([128, IC, XW], f16, tag="pkk", name="pkK")
            if DMA_MODE == "big3":
                nc.gpsimd.dma_start(pkv[:].rearrange("p a b -> p (a b)"), INV[:])
                nc.sync.dma_start(pkq[:].rearrange("p a b -> p (a b)"), INQ[:])
                nc.scalar.dma_start(pkk[:].rearrange("p a b -> p (a b)"), INK[:])
            else:
                rings = (nc.gpsimd, nc.sync, nc.scalar)
                nr = len(rings)
                # V first on every ring, then Q, then K, so each tensor
                # completes as early as possible in that order
                for ti, (pk, IN) in enumerate(((pkv, INV), (pkq, INQ), (pkk, INK))):
                    for ic in range(IC):
                        rings[(ti * IC + ic) % nr].dma_start(
                            pk[:, ic, :], IN[:, ic * XW : (ic + 1) * XW]
                        )
            nc.sync.dma_start(wo_sb[:].rearrange("p a b -> p (a b)"), INW[:])
            nc.scalar.dma_start(sel_sb[:], SEL[:])

            skip_proj = "Y" in phases

            # PSUM (8 banks): scores/proj shared ring 3x[128,1024] (6) + ctx
            # accumulator 1x[65,1024] (2). Projection psums borrow score
            # ring slots so the score stream keeps a 3-deep lookahead.
            with (
                tc.tile_pool(name="attn", bufs=4) as attnp,
                tc.tile_pool(name="psum_s", bufs=3, space="PSUM") as pss,
                tc.tile_pool(name="psum_c", bufs=1, space="PSUM") as psc,
                tc.tile_pool(name="norm", bufs=2) as normp,
            ):
                ctxt_sb = [
                    qkvp.tile([128, S], f16, tag=f"ctxt{ob}", name=f"ctxt{ob}")
                    for ob in range(OB)
                ]

                def proj_qk_group(which, pk, dst, ob, j, ceng):
                    # one [128, 512] column group of the q/k projection
                    ps = pss.tile([128, 512], f32, tag="psS", name=f"ps_{which}{ob}_{j}")
                    for ic in range(IC):
                        nc.tensor.matmul(
                            ps[:],
                            pk[:, ic, S + ob * 128 : S + (ob + 1) * 128],
                            pk[:, ic, j * 512 : (j + 1) * 512],
                            start=(ic == 0),
                            stop=(ic == IC - 1),
                        )
                    if ceng is nc.scalar:
                        ceng.copy(dst[ob][:, j * 512 : (j + 1) * 512], ps[:])
                    else:
                        ceng.tensor_copy(dst[ob][:, j * 512 : (j + 1) * 512], ps[:])

                def proj_v_tb(tb):
                    if skip_proj:
                        return
                    ps = pss.tile([128, 512], f32, tag="psS", name=f"ps_v{tb}")
                    for ic in range(IC):
                        nc.tensor.matmul(
                            ps[:],
                            pkv[:, ic, tb * 128 : (tb + 1) * 128],
                            pkv[:, ic, S : S + O],
                            start=(ic == 0),
                            stop=(ic == IC - 1),
                        )
                    vdst = v_sb[tb].rearrange("p (h x) -> p h x", x=NV)[:, :, 0:D]
                    # Pool/GPSIMD cannot access PSUM (walrus verification)
                    nc.scalar.copy(vdst, ps[:].rearrange("p (h d) -> p h d", d=D))

                # filler queue: remaining q/k projection groups, emitted
                # between attention key blocks to keep the PE stream dense
                fillers = []
                if not skip_proj:
                    for ob in range(1, OB):
                        for which, pk, dst in (("q", pkq, qt_sb), ("k", pkk, kt_sb)):
                            for j in range(S // 512):
                                fillers.append(
                                    (which, pk, dst, ob, j)
                                )
                fill_i = [0]

                def emit_filler():
                    if fill_i[0] < len(fillers):
                        which, pk, dst, ob, j = fillers[fill_i[0]]
                        fill_i[0] += 1
                        proj_qk_group(which, pk, dst, ob, j, nc.vector)

                # Deferred per-chunk finalize (broadcast + normalize mul),
                # emitted early in the NEXT chunk so the waiting instructions
                # never head-of-line-block the PE/DVE queues.
                pending = [None]

                def flush_pending():
                    if pending[0] is not None:
                        fin, pending[0] = pending[0], None
                        fin()

                def attn_head(hl):
                    ob, r0 = hl // 2, (hl % 2) * 64
                    LAG = 3  # ctx matmuls trail scores by LAG key blocks
                    for qc in range(QC):
                        chunk = 2 * hl + qc
                        fill_at = (
                            ()
                            if chunk < 2
                            else (3, 7, 11, 14)
                            if chunk < 4
                            else (4, 9, 14)
                            if chunk < 8
                            else (8,)
                        )
                        q0 = qc * QCHUNK
                        pc = psc.tile([NV, QCHUNK], f32, tag="pc", name=f"pc{hl}_{qc}")
                        at_live = {}
                        for step in range(KB + LAG):
                            kb = step
                            if kb < KB:
                                if hl == 0 and qc == 0:
                                    # warm-up: stream remaining projections
                                    # between the first head's key blocks
                                    if kb in (4, 8, 12):
                                        proj_qk_group("k", pkk, kt_sb, 0, kb // 4, nc.scalar)
                                    if kb == 2:
                                        proj_qk_group("q", pkq, qt_sb, 0, 2, nc.scalar)
                                    if kb == 5:
                                        proj_qk_group("q", pkq, qt_sb, 0, 3, nc.scalar)
                                    proj_v_tb(kb)
                                ps = pss.tile(
                                    [128, QCHUNK], f32, tag="psS", name=f"sc{hl}_{qc}_{kb}"
                                )
                                at = attnp.tile(
                                    [128, QCHUNK], f16, tag="at", name=f"at{hl}_{qc}_{kb}"
                                )
                                for j in range(QCHUNK // 512):
                                    nc.tensor.matmul(
                                        ps[:, j * 512 : (j + 1) * 512],
                                        kt_sb[ob][r0 : r0 + 64, kb * 128 : (kb + 1) * 128],
                                        qt_sb[ob][r0 : r0 + 64, q0 + j * 512 : q0 + (j + 1) * 512],
                                        start=True,
                                        stop=True,
                                    )
                                if kb in DVE_KB:
                                    nc.vector.tensor_scalar(
                                        at[:].bitcast(u16),
                                        ps[:],
                                        A16 * 0.125,
                                        B16,
                                        mybir.AluOpType.mult,
                                        mybir.AluOpType.add,
                                    )
                                else:
                                    nc.scalar.activation(at[:], ps[:], Exp, scale=0.125)
                                at_live[kb] = at
                            ck = step - LAG
                            if ck >= 0:
                                at = at_live.pop(ck)
                                for j in range(QCHUNK // 512):
                                    nc.tensor.matmul(
                                        pc[:, j * 512 : (j + 1) * 512],
                                        v_sb[ck][:, hl * NV : (hl + 1) * NV],
                                        at[:, j * 512 : (j + 1) * 512],
                                        start=(ck == 0),
                                        stop=(ck == KB - 1),
                                    )
                            if step == 1:
                                flush_pending()
                            if step in fill_at:
                                emit_filler()
                        # reciprocal of the denominator row now; broadcast +
                        # multiply deferred into the next chunk
                        rec = normp.tile([1, QCHUNK], f16, tag="rec", name=f"rec{hl}_{qc}")
                        with nc.allow_low_precision(reason="f16 softmax reciprocal"):
                            nc.vector.reciprocal(rec[:], pc[64:65, :])

                        def finalize(pc=pc, rec=rec, q0=q0, hl=hl, ob=ob, r0=r0):
                            # vector ops may read at most one PSUM operand:
                            # stage the ctx rows to SBUF, then multiply by the
                            # broadcast reciprocal still in PSUM
                            stg = normp.tile([64, QCHUNK], f16, tag="stg", name=f"stg{hl}_{q0}", bufs=1)
                            nc.vector.tensor_copy(stg[:], pc[0:64, :])
                            pb = pss.tile([128, QCHUNK], f32, tag="psS", name=f"pb{hl}_{q0}")
                            for j in range(QCHUNK // 512):
                                nc.tensor.matmul(
                                    pb[:, j * 512 : (j + 1) * 512],
                                    sel_sb[0:1, hl % 2, :],
                                    rec[:, j * 512 : (j + 1) * 512],
                                    start=True,
                                    stop=True,
                                )
                            nc.vector.tensor_mul(
                                ctxt_sb[ob][r0 : r0 + 64, q0 : q0 + QCHUNK],
                                stg[:],
                                pb[r0 : r0 + 64, :],
                            )

                        pending[0] = finalize

                if not skip_proj:
                    for j in (0, 1):
                        proj_qk_group("q", pkq, qt_sb, 0, j, nc.scalar)
                    proj_qk_group("k", pkk, kt_sb, 0, 0, nc.scalar)
                if "B" in phases:
                    for hl in range(HL):
                        attn_head(hl)
                    flush_pending()
                else:
                    for tb in range(TB):
                        proj_v_tb(tb)
                    if not skip_proj:
                        for j in (2, 3):
                            proj_qk_group("q", pkq, qt_sb, 0, j, nc.scalar)
                        for j in (1, 2, 3):
                            proj_qk_group("k", pkk, kt_sb, 0, j, nc.scalar)
                while fill_i[0] < len(fillers):
                    emit_filler()

            # ============ Phase D: output projection ============
            with (
                tc.tile_pool(name="outs", bufs=2) as outsp,
                tc.tile_pool(name="psum_o", bufs=2, space="PSUM") as pso,
            ):
                outv = OUT[:].rearrange("(eb p) s -> p eb s", p=128)
                rings = (nc.sync, nc.scalar, nc.gpsimd)
                for eb in range(E // 128 if "D" in phases else 0):
                    po = pso.tile([128, S], f32, tag="po", name=f"po{eb}")
                    for oc in range(OB):
                        for j in range(S // 512):
                            nc.tensor.matmul(
                                po[:, j * 512 : (j + 1) * 512],
                                wo_sb[:, oc, eb * 128 : (eb + 1) * 128],
                                ctxt_sb[oc][:, j * 512 : (j + 1) * 512],
                                start=(oc == 0),
                                stop=(oc == OB - 1),
                            )
                    so = outsp.tile([128, S], f16, tag="so", name=f"so{eb}")
                    nc.scalar.copy(so[:], po[:])
                    rings[eb % 3].dma_start(outv[:, eb, :], so[:])

    nc.compile()
    return nc


def _get_nc():
    if "nc" not in _CACHE:
        _CACHE["nc"] = _build()
    return _CACHE["nc"]


def _shard_inputs(Q, K, V, Wq, Wk, Wv, Wo):
    f16 = np.float16
    Q = np.asarray(Q, np.float32)
    K = np.asarray(K, np.float32)
    V = np.asarray(V, np.float32)
    Wq = np.asarray(Wq, np.float32)
    Wk = np.asarray(Wk, np.float32)
    Wv = np.asarray(Wv, np.float32)
    Wo = np.asarray(Wo, np.float32)

    sel = np.zeros((1, 2, 128), np.float32)
    sel[0, 0, 0:64] = 1.0
    sel[0, 1, 64:128] = 1.0
    sel = sel.astype(f16)

    def pack_xw(XTb, WTg):
        # [128, IC*(S+O)]: per chunk ic, activation chunk then weight chunk
        parts = []
        for ic in range(IC):
            parts.append(XTb[ic * 128 : (ic + 1) * 128, :])
            parts.append(WTg[ic * 128 : (ic + 1) * 128, :])
        return np.ascontiguousarray(np.concatenate(parts, axis=1)).astype(f16)

    in_maps = []
    for c in range(N_CORES):
        b, g = divmod(c, 2)
        sl = slice(g * O, (g + 1) * O)
        wot = Wo[:, sl].T  # [O, E]
        inw = np.concatenate(
            [wot[oc * 128 : (oc + 1) * 128, :] for oc in range(OB)], axis=1
        )
        in_maps.append(
            {
                "SEL": sel,
                "INQ": pack_xw(Q[b].T, Wq[sl, :].T),
                "INK": pack_xw(K[b].T, Wk[sl, :].T),
                "INV": pack_xw(V[b].T, Wv[sl, :].T),
                "INW": np.ascontiguousarray(inw).astype(f16),
            }
        )
    return in_maps


class _Runner:
    """Compile-once executor for the SPMD bass program on 8 cores."""

    def __init__(self, nc, donate=True):
        import jax
        import concourse.mybir as mybir
        from concourse import bass2jax

        bass2jax.install_neuronx_cc_hook()
        self.jax = jax
        self.nc = nc
        partition_name = (
            nc.partition_id_tensor.name if nc.partition_id_tensor else None
        )
        in_names, out_names, out_avals = [], [], []
        for alloc in nc.m.functions[0].allocations:
            if not isinstance(alloc, mybir.MemoryLocationSet):
                continue
            name = alloc.memorylocations[0].name
            if alloc.kind == "ExternalInput":
                if name != partition_name:
                    in_names.append(name)
            elif alloc.kind == "ExternalOutput":
                out_names.append(name)
                out_avals.append(
                    jax.core.ShapedArray(
                        tuple(alloc.tensor_shape), mybir.dt.np(alloc.dtype)
                    )
                )
        self.in_names = in_names
        self.out_names = out_names
        self.out_avals = out_avals
        n_params = len(in_names)
        n_outs = len(out_names)
        all_in_names = list(in_names) + list(out_names)
        if partition_name is not None:
            all_in_names.append(partition_name)
        all_in_names = tuple(all_in_names)

        def _body(*args):
            operands = list(args)
            if partition_name is not None:
                operands.append(bass2jax.partition_id_tensor())
            outs = bass2jax._bass_exec_p.bind(
                *operands,
                out_avals=tuple(out_avals),
                in_names=all_in_names,
                out_names=tuple(out_names),
                lowering_input_output_aliases=(),
                sim_require_finite=True,
                sim_require_nnan=True,
                nc=nc,
            )
            return tuple(outs)

        from concourse.bass2jax import Mesh, PartitionSpec, shard_map

        devices = jax.devices()[:N_CORES]
        mesh = Mesh(np.asarray(devices), ("core",))
        self.mesh = mesh
        self.pspec = PartitionSpec("core")
        self.sharded = jax.jit(
            shard_map(
                _body,
                mesh=mesh,
                in_specs=(PartitionSpec("core"),) * (n_params + n_outs),
                out_specs=(PartitionSpec("core"),) * n_outs,
                check_rep=False,
            ),
            donate_argnums=(
                tuple(range(n_params, n_params + n_outs)) if donate else ()
            ),
            keep_unused=True,
        )

    def concat_inputs(self, in_maps):
        return [
            np.concatenate([np.asarray(m[name]) for m in in_maps], axis=0)
            for name in self.in_names
        ]

    def zero_outs(self):
        return [
            np.zeros((N_CORES * a.shape[0], *a.shape[1:]), a.dtype)
            for a in self.out_avals
        ]

    def __call__(self, concat_in, concat_zeros=None):
        if concat_zeros is None:
            concat_zeros = self.zero_outs()
        out_arrs = self.sharded(*concat_in, *concat_zeros)
        return [
            {
                name: np.asarray(out_arrs[i]).reshape(
                    N_CORES, *self.out_avals[i].shape
                )[c]
                for i, name in enumerate(self.out_names)
            }
            for c in range(N_CORES)
        ]


def _get_runner():
    if "runner" not in _CACHE:
        _CACHE["runner"] = _Runner(_get_nc())
    return _CACHE["runner"]


def kernel(Q, K, V, mask, Wq, Wk, Wv, Wo):
    runner = _get_runner()
    in_maps = _shard_inputs(Q, K, V, Wq, Wk, Wv, Wo)
    results = runner(runner.concat_inputs(in_maps))
    out = np.empty((B, S, E), np.float32)
    for b in range(B):
        acc = results[2 * b]["OUT"].astype(np.float32) + results[2 * b + 1][
            "OUT"
        ].astype(np.float32)
        out[b] = acc.T
    return out


# revision 49
# speedup vs baseline: 1.2878x; 1.1854x over previous
"""Trainium2 Bass kernel for 16-head MHA (B=4, S=2048, E=1024), 8 NeuronCores.

Sharding: core c handles batch b = c//2 and head-group g = c%2 (8 heads each).
Column-parallel Wq/Wk/Wv, row-parallel Wo; the two partial Wo outputs per
batch are summed on the host.

All matmuls are fp16 with fp32 PSUM accumulation (fp8 variants tested ~3-10x
over the accuracy budget). The performance levers vs the naive schedule:
  - The PE ramps to full clock only after ~3us of gapless execution, so the
    emission order keeps PE dense: V/Q/K projections stream through two
    40KB pack buffers, remaining Q/K projection chunks are interleaved as
    filler groups between attention key blocks.
  - exp is split across ScalarE (12/16 tiles, exact Exp) and DVE (4/16
    tiles, Schraudolph bitcast-exp: one tensor_scalar affine fp32->uint16
    whose bits are the f16 exponential; ~+-3% on those tiles only), sized so
    the exp stream stays just under the PE's B-phase time.
  - Per (head, 1024-query chunk) the softmax normalization runs straight
    out of PSUM: DVE reciprocal of the denominator row (ones-column trick),
    1-partition matmul broadcast, DVE stage+multiply into ctxt -- all
    deferred into the next chunk so waiting instructions never block the
    in-order engine queues.
  - Phase D staged per-eb with copies on ScalarE and outputs shipped on 3
    DMA rings as they finish. (Pool/GPSIMD cannot touch PSUM, and vector
    ops may read at most one PSUM operand -- walrus-verified rules.)
"""

import sys

sys.path.insert(0, "/opt/trn_rl_repo")

import numpy as np

# Problem constants (hardcoded; kernel.py must be self-contained).
B = 4
S = 2048
E = 1024
H = 16
D = 64
N_CORES = 8
HL = H // 2  # heads per core
O = HL * D  # 512 local out features of q/k/v projections
IC = E // 128  # 8 contraction chunks
OB = O // 128  # 4 output row-blocks (head pairs)
TB = S // 128  # 16 token blocks
KB = S // 128  # 16 key blocks per head
QCHUNK = 1024
QC = S // QCHUNK  # 2
NV = D + 1  # v dims + ones column

DVE_KB = (3, 7, 11, 15)  # key blocks whose exp runs on DVE (Schraudolph)
A16 = 1024.0 / np.log(2.0)  # Schraudolph slope for f16 bits
# f16 exponent offset, minus the PWL centering constant, plus 0.5 because the
# DVE float->uint convert truncates (verified) rather than rounds.
B16 = 15.0 * 1024.0 - 60.0 + 0.5

_CACHE = {}

DMA_MODE = "chunk3"  # "big3" | "chunk3"


def _build(phases="ABCD"):
    import concourse.bass as bass
    import concourse.mybir as mybir
    from concourse import bacc, tile

    f32 = mybir.dt.float32
    f16 = mybir.dt.float16
    u16 = mybir.dt.uint16
    Exp = mybir.ActivationFunctionType.Exp

    nc = bacc.Bacc(None, target_bir_lowering=False)

    # Packed inputs: per contraction chunk ic, activation chunk [128, S] then
    # projection-weight chunk [128, O]; consumed V -> Q -> K through two pack
    # buffers.
    XW = S + O
    INQ = nc.dram_tensor("INQ", [128, IC * XW], f16, kind="ExternalInput")
    INK = nc.dram_tensor("INK", [128, IC * XW], f16, kind="ExternalInput")
    INV = nc.dram_tensor("INV", [128, IC * XW], f16, kind="ExternalInput")
    INW = nc.dram_tensor("INW", [128, OB * E], f16, kind="ExternalInput")
    SEL = nc.dram_tensor("SEL", [1, 2, 128], f16, kind="ExternalInput")
    OUT = nc.dram_tensor("OUT", [E, S], f16, kind="ExternalOutput")

    with tile.TileContext(nc) as tc:
        with (
            tc.tile_pool(name="consts", bufs=1) as constp,
            tc.tile_pool(name="weights", bufs=1) as wp,
            tc.tile_pool(name="qkv", bufs=1) as qkvp,
            tc.tile_pool(name="pack", bufs=1) as packp,
        ):
            sel_sb = constp.tile([1, 2, 128], f16, tag="sel")
            wo_sb = wp.tile([128, OB, E], f16, tag="wo")

            qt_sb = [qkvp.tile([128, S], f16, tag=f"qt{ob}", name=f"qt{ob}") for ob in range(OB)]
            kt_sb = [qkvp.tile([128, S], f16, tag=f"kt{ob}", name=f"kt{ob}") for ob in range(OB)]
            v_sb = [qkvp.tile([128, HL * NV], f16, tag=f"v{tb}", name=f"v{tb}") for tb in range(TB)]
            for tb in range(TB):
                ones = v_sb[tb].rearrange("p (h x) -> p h x", x=NV)[:, :, D : D + 1]
                nc.vector.memset(ones, 1.0)

            # Per-chunk DMAs: subtile dep tracking lets the first projection
            # matmul start as soon as its ic-chunk lands, not the whole pack.
            pkv = packp.tile([128, IC, XW], f16, tag="pkv", name="pkV")
            pkq = packp.tile([128, IC, XW], f16, tag="pkq", name="pkQ")
            pkk = packp.tile([128, IC, XW], f16, tag="pkk", name="pkK")
            if DMA_MODE == "big3":
                nc.gpsimd.dma_start(pkv[:].rearrange("p a b -> p (a b)"), INV[:])
                nc.sync.dma_start(pkq[:].rearrange("p a b -> p (a b)"), INQ[:])
                nc.scalar.dma_start(pkk[:].rearrange("p a b -> p (a b)"), INK[:])
            else:
                rings = (nc.gpsimd, nc.sync, nc.scalar)
                nr = len(rings)
                # V first on every ring, then Q, then K, so each tensor
                # completes as early as possible in that order
                for ti, (pk, IN) in enumerate(((pkv, INV), (pkq, INQ), (pkk, INK))):
                    for ic in range(IC):
                        rings[(ti * IC + ic) % nr].dma_start(
                            pk[:, ic, :], IN[:, ic * XW : (ic + 1) * XW]
                        )
            nc.sync.dma_start(wo_sb[:].rearrange("p a b -> p (a b)"), INW[:])
            nc.scalar.dma_start(sel_sb[:], SEL[:])

            skip_proj = "Y" in phases

            # PSUM (8 banks): scores/proj shared ring 3x[128,1024] (6) + ctx
            # accumulator 1x[65,1024] (2). Projection psums borrow score
            # ring slots so the score stream keeps a 3-deep lookahead.
            with (
                tc.tile_pool(name="attn", bufs=4) as attnp,
                tc.tile_pool(name="psum_s", bufs=3, space="PSUM") as pss,
                tc.tile_pool(name="psum_c", bufs=1, space="PSUM") as psc,
                tc.tile_pool(name="norm", bufs=2) as normp,
            ):
                ctxt_sb = [
                    qkvp.tile([128, S], f16, tag=f"ctxt{ob}", name=f"ctxt{ob}")
                    for ob in range(OB)
                ]

                def proj_qk_group(which, pk, dst, ob, j, ceng):
                    # one [128, 512] column group of the q/k projection
                    ps = pss.tile([128, 512], f32, tag="psS", name=f"ps_{which}{ob}_{j}")
                    for ic in range(IC):
                        nc.tensor.matmul(
                            ps[:],
                            pk[:, ic, S + ob * 128 : S + (ob + 1) * 128],
                            pk[:, ic, j * 512 : (j + 1) * 512],
                            start=(ic == 0),
                            stop=(ic == IC - 1),
                        )
                    if ceng is nc.scalar:
                        ceng.copy(dst[ob][:, j * 512 : (j + 1) * 512], ps[:])
                    else:
                        ceng.tensor_copy(dst[ob][:, j * 512 : (j + 1) * 512], ps[:])

                def proj_v_tb(tb):
                    if skip_proj:
                        return
                    ps = pss.tile([128, 512], f32, tag="psS", name=f"ps_v{tb}")
                    for ic in range(IC):
                        nc.tensor.matmul(
                            ps[:],
                            pkv[:, ic, tb * 128 : (tb + 1) * 128],
                            pkv[:, ic, S : S + O],
                            start=(ic == 0),
                            stop=(ic == IC - 1),
                        )
                    vdst = v_sb[tb].rearrange("p (h x) -> p h x", x=NV)[:, :, 0:D]
                    # Pool/GPSIMD cannot access PSUM (walrus verification)
                    nc.scalar.copy(vdst, ps[:].rearrange("p (h d) -> p h d", d=D))

                # filler queue: remaining q/k projection groups, emitted
                # between attention key blocks to keep the PE stream dense
                fillers = []
                if not skip_proj:
                    for ob in range(1, OB):
                        for which, pk, dst in (("q", pkq, qt_sb), ("k", pkk, kt_sb)):
                            for j in range(S // 512):
                                fillers.append(
                                    (which, pk, dst, ob, j)
                                )
                fill_i = [0]

                def emit_filler():
                    if fill_i[0] < len(fillers):
                        which, pk, dst, ob, j = fillers[fill_i[0]]
                        fill_i[0] += 1
                        proj_qk_group(which, pk, dst, ob, j, nc.vector)

                # Deferred per-chunk finalize (broadcast + normalize mul),
                # emitted early in the NEXT chunk so the waiting instructions
                # never head-of-line-block the PE/DVE queues.
                pending = [None]

                def flush_pending():
                    if pending[0] is not None:
                        fin, pending[0] = pending[0], None
                        fin()

                def attn_head(hl):
                    ob, r0 = hl // 2, (hl % 2) * 64
                    LAG = 3  # ctx matmuls trail scores by LAG key blocks
                    for qc in range(QC):
                        chunk = 2 * hl + qc
                        fill_at = (
                            ()
                            if chunk < 2
                            else (3, 7, 11, 14)
                            if chunk < 4
                            else (4, 9, 14)
                            if chunk < 8
                            else (8,)
                        )
                        q0 = qc * QCHUNK
                        pc = psc.tile([NV, QCHUNK], f32, tag="pc", name=f"pc{hl}_{qc}")
                        at_live = {}
                        for step in range(KB + LAG):
                            kb = step
                            if kb < KB:
                                if hl == 0 and qc == 0:
                                    # warm-up: stream remaining projections
                                    # between the first head's key blocks
                                    if kb in (4, 8, 12):
                                        proj_qk_group("k", pkk, kt_sb, 0, kb // 4, nc.scalar)
                                    if kb == 2:
                                        proj_qk_group("q", pkq, qt_sb, 0, 2, nc.scalar)
                                    if kb == 5:
                                        proj_qk_group("q", pkq, qt_sb, 0, 3, nc.scalar)
                                    proj_v_tb(kb)
                                ps = pss.tile(
                                    [128, QCHUNK], f32, tag="psS", name=f"sc{hl}_{qc}_{kb}"
                                )
                                at = attnp.tile(
                                    [128, QCHUNK], f16, tag="at", name=f"at{hl}_{qc}_{kb}"
                                )
                                for j in range(QCHUNK // 512):
                                    nc.tensor.matmul(
                                        ps[:, j * 512 : (j + 1) * 512],
                                        kt_sb[ob][r0 : r0 + 64, kb * 128 : (kb + 1) * 128],
                                        qt_sb[ob][r0 : r0 + 64, q0 + j * 512 : q0 + (j + 1) * 512],
                                        start=True,
                                        stop=True,
                                    )
                                if kb in DVE_KB:
                                    nc.vector.tensor_scalar(
                                        at[:].bitcast(u16),
                                        ps[:],
                                        A16 * 0.125,
                                        B16,
                                        mybir.AluOpType.mult,
                                        mybir.AluOpType.add,
                                    )
                                else:
                                    nc.scalar.activation(at[:], ps[:], Exp, scale=0.125)
                                at_live[kb] = at
                            ck = step - LAG
                            if ck >= 0:
                                at = at_live.pop(ck)
                                for j in range(QCHUNK // 512):
                                    nc.tensor.matmul(
                                        pc[:, j * 512 : (j + 1) * 512],
                                        v_sb[ck][:, hl * NV : (hl + 1) * NV],
                                        at[:, j * 512 : (j + 1) * 512],
                                        start=(ck == 0),
                                        stop=(ck == KB - 1),
                                    )
                            if step == 1:
                                flush_pending()
                            if step in fill_at:
                                emit_filler()
                        # reciprocal of the denominator row now; broadcast +
                        # multiply deferred into the next chunk
                        rec = normp.tile([1, QCHUNK], f16, tag="rec", name=f"rec{hl}_{qc}")
                        with nc.allow_low_precision(reason="f16 softmax reciprocal"):
                            nc.vector.reciprocal(rec[:], pc[64:65, :])

                        def finalize(pc=pc, rec=rec, q0=q0, hl=hl, ob=ob, r0=r0):
                            # vector ops may read at most one PSUM operand:
                            # stage the ctx rows to SBUF, then multiply by the
                            # broadcast reciprocal still in PSUM
                            stg = normp.tile([64, QCHUNK], f16, tag="stg", name=f"stg{hl}_{q0}", bufs=1)
                            nc.vector.tensor_copy(stg[:], pc[0:64, :])
                            pb = pss.tile([128, QCHUNK], f32, tag="psS", name=f"pb{hl}_{q0}")
                            for j in range(QCHUNK // 512):
                                nc.tensor.matmul(
                                    pb[:, j * 512 : (j + 1) * 512],
                                    sel_sb[0:1, hl % 2, :],
                                    rec[:, j * 512 : (j + 1) * 512],
                                    start=True,
                                    stop=True,
                                )
                            nc.vector.tensor_mul(
                                ctxt_sb[ob][r0 : r0 + 64, q0 : q0 + QCHUNK],
                                stg[:],
                                pb[r0 : r0 + 64, :],
                            )

                        pending[0] = finalize

                if not skip_proj:
                    for j in (0, 1):
                        proj_qk_group("q", pkq, qt_sb, 0, j, nc.scalar)
                    proj_qk_group("k", pkk, kt_sb, 0, 0, nc.scalar)
                if "B" in phases:
                    for hl in range(HL):
                        attn_head(hl)
                    flush_pending()
                else:
                    for tb in range(TB):
                        proj_v_tb(tb)
                    if not skip_proj:
                        for j in (2, 3):
                            proj_qk_group("q", pkq, qt_sb, 0, j, nc.scalar)
                        for j in (1, 2, 3):
                            proj_qk_group("k", pkk, kt_sb, 0, j, nc.scalar)
                while fill_i[0] < len(fillers):
                    emit_filler()

            # ============ Phase D: output projection ============
            with (
                tc.tile_pool(name="outs", bufs=2) as outsp,
                tc.tile_pool(name="psum_o", bufs=2, space="PSUM") as pso,
            ):
                outv = OUT[:].rearrange("(eb p) s -> p eb s", p=128)
                rings = (nc.sync, nc.scalar, nc.gpsimd)
                for eb in range(E // 128 if "D" in phases else 0):
                    po = pso.tile([128, S], f32, tag="po", name=f"po{eb}")
                    for oc in range(OB):
                        for j in range(S // 512):
                            nc.tensor.matmul(
                                po[:, j * 512 : (j + 1) * 512],
                                wo_sb[:, oc, eb * 128 : (eb + 1) * 128],
                                ctxt_sb[oc][:, j * 512 : (j + 1) * 512],
                                start=(oc == 0),
                                stop=(oc == OB - 1),
                            )
                    so = outsp.tile([128, S], f16, tag="so", name=f"so{eb}")
                    nc.scalar.copy(so[:], po[:])
                    rings[eb % 3].dma_start(outv[:, eb, :], so[:])

    nc.compile()
    return nc


def _get_nc():
    if "nc" not in _CACHE:
        _CACHE["nc"] = _build()
    return _CACHE["nc"]


def _shard_inputs(Q, K, V, Wq, Wk, Wv, Wo):
    f16 = np.float16
    Q = np.asarray(Q, np.float32)
    K = np.asarray(K, np.float32)
    V = np.asarray(V, np.float32)
    Wq = np.asarray(Wq, np.float32)
    Wk = np.asarray(Wk, np.float32)
    Wv = np.asarray(Wv, np.float32)
    Wo = np.asarray(Wo, np.float32)

    sel = np.zeros((1, 2, 128), np.float32)
    sel[0, 0, 0:64] = 1.0
    sel[0, 1, 64:128] = 1.0
    sel = sel.astype(f16)

    def pack_xw(XTb, WTg):
        # [128, IC*(S+O)]: per chunk ic, activation chunk then weight chunk
        parts = []
        for ic in range(IC):
            parts.append(XTb[ic * 128 : (ic + 1) * 128, :])
            parts.append(WTg[ic * 128 : (ic + 1) * 128, :])
        return np.ascontiguousarray(np.concatenate(parts, axis=1)).astype(f16)

    in_maps = []
    for c in range(N_CORES):
        b, g = divmod(c, 2)
        sl = slice(g * O, (g + 1) * O)
        wot = Wo[:, sl].T  # [O, E]
        inw = np.concatenate(
            [wot[oc * 128 : (oc + 1) * 128, :] for oc in range(OB)], axis=1
        )
        in_maps.append(
            {
                "SEL": sel,
                "INQ": pack_xw(Q[b].T, Wq[sl, :].T),
                "INK": pack_xw(K[b].T, Wk[sl, :].T),
                "INV": pack_xw(V[b].T, Wv[sl, :].T),
                "INW": np.ascontiguousarray(inw).astype(f16),
            }
        )
    return in_maps


class _Runner:
    """Compile-once executor for the SPMD bass program on 8 cores."""

    def __init__(self, nc, donate=True):
        import jax
        import concourse.mybir as mybir
        from concourse import bass2jax

        bass2jax.install_neuronx_cc_hook()
        self.jax = jax
        self.nc = nc
        partition_name = (
            nc.partition_id_tensor.name if nc.partition_id_tensor else None
        )
        in_names, out_names, out_avals = [], [], []
        for alloc in nc.m.functions[0].allocations:
            if not isinstance(alloc, mybir.MemoryLocationSet):
                continue
            name = alloc.memorylocations[0].name
            if alloc.kind == "ExternalInput":
                if name != partition_name:
                    in_names.append(name)
            elif alloc.kind == "ExternalOutput":
                out_names.append(name)
                out_avals.append(
                    jax.core.ShapedArray(
                        tuple(alloc.tensor_shape), mybir.dt.np(alloc.dtype)
                    )
                )
        self.in_names = in_names
        self.out_names = out_names
        self.out_avals = out_avals
        n_params = len(in_names)
        n_outs = len(out_names)
        all_in_names = list(in_names) + list(out_names)
        if partition_name is not None:
            all_in_names.append(partition_name)
        all_in_names = tuple(all_in_names)

        def _body(*args):
            operands = list(args)
            if partition_name is not None:
                operands.append(bass2jax.partition_id_tensor())
            outs = bass2jax._bass_exec_p.bind(
                *operands,
                out_avals=tuple(out_avals),
                in_names=all_in_names,
                out_names=tuple(out_names),
                lowering_input_output_aliases=(),
                sim_require_finite=True,
                sim_require_nnan=True,
                nc=nc,
            )
            return tuple(outs)

        from concourse.bass2jax import Mesh, PartitionSpec, shard_map

        devices = jax.devices()[:N_CORES]
        mesh = Mesh(np.asarray(devices), ("core",))
        self.mesh = mesh
        self.pspec = PartitionSpec("core")
        self.sharded = jax.jit(
            shard_map(
                _body,
                mesh=mesh,
                in_specs=(PartitionSpec("core"),) * (n_params + n_outs),
                out_specs=(PartitionSpec("core"),) * n_outs,
                check_rep=False,
            ),
            donate_argnums=(
                tuple(range(n_params, n_params + n_outs)) if donate else ()
            ),
            keep_unused=True,
        )

    def concat_inputs(self, in_maps):
        return [
            np.concatenate([np.asarray(m[name]) for m in in_maps], axis=0)
            for name in self.in_names
        ]

    def zero_outs(self):
        return [
            np.zeros((N_CORES * a.shape[0], *a.shape[1:]), a.dtype)
            for a in self.out_avals
        ]

    def __call__(self, concat_in, concat_zeros=None):
        if concat_zeros is None:
            concat_zeros = self.zero_outs()
        out_arrs = self.sharded(*concat_in, *concat_zeros)
        return [
            {
                name: np.asarray(out_arrs[i]).reshape(
                    N_CORES, *self.out_avals[i].shape
                )[c]
                for i, name in enumerate(self.out_names)
            }
            for c in range(N_CORES)
        ]


def _get_runner():
    if "runner" not in _CACHE:
        _CACHE["runner"] = _Runner(_get_nc())
    return _CACHE["runner"]


def kernel(Q, K, V, mask, Wq, Wk, Wv, Wo):
    runner = _get_runner()
    in_maps = _shard_inputs(Q, K, V, Wq, Wk, Wv, Wo)
    results = runner(runner.concat_inputs(in_maps))
    out = np.empty((B, S, E), np.float32)
    for b in range(B):
        acc = results[2 * b]["OUT"].astype(np.float32) + results[2 * b + 1][
            "OUT"
        ].astype(np.float32)
        out[b] = acc.T
    return out
